# revision 37
# baseline (speedup 1.0000x reference)
"""Trainium2 Bass kernel for softmax-free attention:
    q = x @ Wq^T; k = x @ Wk^T; v = x @ Wv^T
    s = (q @ k^T) / sqrt(d); out = s @ v
  x: [4, 4096, 1024], W*: [1024, 1024], out: [4, 4096, 1024] (fp32)

KEY RESTRUCTURE: there is no softmax, so the chain is associative:
    out = q (k^T v) / sqrt(d) = x @ [Wq^T Wk (x^T x) Wv^T] / sqrt(d)
With G_b = x_b^T x_b (per batch), the per-core FLOPs drop from ~47 GF
(score-matrix path, PE-bound ~600us) to ~12 GF:
    G  = x_own^T x_own  (+ pair exchange over the 2 cores of a batch)
    AT = A^T where A = Wq^T Wk                             [input-only]
    M  = G @ WvT'[:, own d-half]   (WvT' = Wv^T/sqrt(d), host-folded)
    N  = A @ M                      (column split propagates for free)
    out = x_own @ [N_own | N_peer]
M and N are computed only for this core's 512 d-columns; the 1MB
N-half exchange rides the same Shared-DRAM + token-AllReduce machinery
as the G exchange and hides under the out phase's own-half matmuls.
WvT' columns are rotated per-core on the host so "own half" is always
columns 0:512 (SPMD program stays identical across cores); the host
un-rotates the output columns after gather.

All on-chip operands are bf16 (same 78.6 TF/s PE rate as fp32r, ~5e-3
end-to-end rel err vs the 2e-2 gate), fp32 PSUM accumulation, 8-16
deep groups into [128,512] tiles. All input loads are serialized on
the sync ring behind xn so G (the first PE phase) is never starved;
the HAM warmup covers the xn load latency.

Exchange ordering: a DRAM-to-DRAM token DMA samples the Shared spill
region (RAW dep on all spill writes via the dynamic-slot alias)
straight into the collective's input; the tiny pair AllReduce
completes only when both members' spills are durable.  The AT phase
(no barrier dependency) covers the collective's latency; the peer-G
column-half adds are emitted on the DVE only after the whole AT copy
stream, so a late barrier can never stall the AT copies or PSUM
recycling.  Gated peer reads stay off the sync ring (the collective
machinery contends with it); output stores ride the scalar ring with
the gated peer-N read emitted ahead of them.
"""

import sys
import types
from contextlib import ExitStack

import numpy as np
from ml_dtypes import bfloat16

import concourse.bass as bass
import concourse.tile as tile
from concourse import bacc, mybir
from concourse.bass_utils import run_bass_kernel_spmd
from concourse.mybir import EngineType
from concourse.tile import add_dep_helper
from concourse.vector_clock import ScopedClock

# ---------------------------------------------------------------------------
# Environment shims
# ---------------------------------------------------------------------------


def _install_tile_drain_patch():
    """This toolchain's walrus caps sync waits at 1 per instruction, but
    TileContext's tail drain can carry several. Split the overflow onto
    preceding nops (same semantics: the issuing engine observes every sem
    before draining)."""
    if getattr(tile.TileContext, "_drain_patch_installed", False):
        return

    def _patched_drain_and_barrier(self, tick_clock, wait_clock):
        nc = self.nc
        collector = nc.sync.nop(hint="drain_wait_collector", nofuse=True)
        wait_clock.add_sem_waits(
            collector.ins, ScopedClock({None: tick_clock.global_clock})
        )
        waits = list(collector.ins.sync_info.on_wait or [])
        if len(waits) > 1:
            collector.ins.sync_info.on_wait = [waits[0]]
            for w in waits[1:]:
                nop = nc.sync.nop(hint="drain_wait_extra", nofuse=True)
                nop.ins.sync_info = mybir.SyncInfo(on_wait=[w], on_update=[])
        nc.sync.drain()

        nc.all_engine_barrier()
        assert self.sems is not None
        popped = nc._tile_sem_poison_stack.pop()
        assert popped is self._sem_poison
        nc.clear_and_free_semaphores(list(self.sems.allocated().values()))
        nc.all_engine_barrier()

    tile.TileContext._drain_and_barrier = _patched_drain_and_barrier
    tile.TileContext._drain_patch_installed = True


def _install_ntff_shim():
    """The image's antenv lacks axon_hooks, which silently degrades
    trace=True. Recreate the get/set pair and register the ctypes NTFF hook
    from trn_agent_boot (no-op if unavailable)."""
    if "antenv.axon_hooks" in sys.modules:
        return
    state = {"hook": None}

    def set_axon_ntff_profile_hook(h):
        state["hook"] = h

    def get_axon_ntff_profile_hook():
        return state["hook"]

    mod = types.ModuleType("antenv.axon_hooks")
    mod.set_axon_ntff_profile_hook = set_axon_ntff_profile_hook
    mod.get_axon_ntff_profile_hook = get_axon_ntff_profile_hook
    sys.modules["antenv.axon_hooks"] = mod
    try:
        import antenv

        antenv.axon_hooks = mod
        from trn_agent_boot.trn_boot import _ntff_profile_via_ctypes

        set_axon_ntff_profile_hook(
            _ntff_profile_via_ctypes("/opt/axon/libaxon_pjrt.so")
        )
    except Exception:
        pass


_install_tile_drain_patch()
_install_ntff_shim()

# ---------------------------------------------------------------------------
# Problem constants (hardcoded per the harness contract)
# ---------------------------------------------------------------------------

B, L, D = 4, 4096, 1024
N_CORES = 8
P = 128
LH = L // 2          # rows per core
DC = D // P          # 8 chunks of 128 over d/e/c/a/f
LC = LH // P         # 16 l-chunks of the natural-layout x
FREE = 512           # PSUM tile free dim (one bank, fp32 accum)
F32 = mybir.dt.float32
BF16 = mybir.dt.bfloat16
PAIRS = [[2 * i, 2 * i + 1] for i in range(N_CORES // 2)]
WU_GROUPS = 14       # HAM warmup groups (~12us at cold clock)


def build_nc():
    nc = bacc.Bacc("TRN2", target_bir_lowering=False, debug=False,
                   num_devices=N_CORES)
    xn = nc.dram_tensor("xn", [LH, D], BF16, kind="ExternalInput").ap()
    xT = nc.dram_tensor("xT", [D, LH], BF16, kind="ExternalInput").ap()
    wq = nc.dram_tensor("wq", [D, D], BF16, kind="ExternalInput").ap()
    wk = nc.dram_tensor("wk", [D, D], BF16, kind="ExternalInput").ap()
    wvT = nc.dram_tensor("wvT", [D, FREE], BF16, kind="ExternalInput").ap()
    out = nc.dram_tensor("out", [LH, D], F32, kind="ExternalOutput").ap()
    slots = nc.dram_tensor("slots", [1, 2], mybir.dt.uint32,
                           kind="ExternalInput").ap()
    Gsh = nc.dram_tensor("Gsh", [2, D, D], BF16, addr_space="Shared").ap()
    Nsh = nc.dram_tensor("Nsh", [2, D, FREE], BF16, addr_space="Shared").ap()
    tok = nc.dram_tensor("tok", [1, 2], BF16).ap()
    tok2 = nc.dram_tensor("tok2", [1, 2], BF16).ap()
    tok3 = nc.dram_tensor("tok3", [1, 2], BF16).ap()
    tok4 = nc.dram_tensor("tok4", [1, 2], BF16).ap()
    tokd = nc.dram_tensor("tokd", [1, 2], BF16).ap()
    tokd2 = nc.dram_tensor("tokd2", [1, 2], BF16).ap()
    wu_sink = nc.dram_tensor("wu_sink", [P, FREE], F32).ap()

    def chunked(ap):  # [K*, N] dram -> [P, K*/P, N] partition-major
        return ap.rearrange("(c p) n -> p c n", p=P)

    with tile.TileContext(nc) as tc, ExitStack() as octx:
        psum = octx.enter_context(tc.tile_pool(name="psum", bufs=8, space="PSUM"))
        tokp = octx.enter_context(tc.tile_pool(name="tokp", bufs=1))

        # HAM warmup: junk matmuls while xn loads, so the PE clock gate is
        # already at 8/8 when real work arrives
        wut = tokp.tile([P, FREE], BF16, tag="wut")
        nc.vector.memset(wut[:].bitcast(mybir.dt.uint16), 0)
        wuo = tokp.tile([P, FREE], F32, tag="wuo")
        for g in range(WU_GROUPS):
            wp = psum.tile([P, FREE], F32, tag="ps", name=f"wu_{g}")
            for r in range(2):
                nc.tensor.matmul(wp[:], wut[:, 0:P], wut[:],
                                 start=(r == 0), stop=(r == 1))
            if g == WU_GROUPS - 1:
                nc.vector.tensor_copy(wuo[:], wp[:])
        nc.gpsimd.dma_start(wu_sink[:], wuo[:])

        # slot registers (first on the sync ring: 8 bytes)
        st_sl = tokp.tile([1, 2], mybir.dt.uint32, tag="sl", bufs=1)
        nc.sync.dma_start(st_sl[:], slots[:])
        regs_o = nc.alloc_registers(
            engines=[EngineType.SP, EngineType.Activation])
        nc.regs_load(regs_o, st_sl[0:1, 0:1])
        svo = nc.snap(regs_o, donate=True)
        regs_p = nc.alloc_registers(
            engines=[EngineType.SP, EngineType.Activation])
        nc.regs_load(regs_p, st_sl[0:1, 1:2])
        svp = nc.snap(regs_p, donate=True)

        # persistent SBUF tiles; ALL input loads serialized on the sync ring
        # in need-order (xn feeds the first PE phase)
        xbig_pool = octx.enter_context(tc.tile_pool(name="xbig", bufs=1))
        xnt = xbig_pool.tile([P, LC, D], BF16, tag="xb")  # xn natural
        for h in range(4):
            nc.sync.dma_start(xnt[:, 4 * h:4 * (h + 1)],
                              chunked(xn)[:, 4 * h:4 * (h + 1)])
        wpool = octx.enter_context(tc.tile_pool(name="wpool", bufs=1))
        wqt = wpool.tile([P, DC, D], BF16, tag="wq")
        wkt = wpool.tile([P, DC, D], BF16, tag="wk")
        wvt = wpool.tile([P, DC, FREE], BF16, tag="wv")
        nc.sync.dma_start(wkt[:], chunked(wk))
        nc.sync.dma_start(wqt[:], chunked(wq))
        nc.sync.dma_start(wvt[:], chunked(wvT))
        gpool = octx.enter_context(tc.tile_pool(name="gpool", bufs=1))
        gsb = gpool.tile([P, DC, D], BF16, tag="g")
        atsb = gpool.tile([P, DC, D], BF16, tag="at")
        msb = gpool.tile([P, DC, FREE], BF16, tag="m")
        nsb = gpool.tile([P, DC, D], BF16, tag="n")
        gpp = octx.enter_context(tc.tile_pool(name="gpp", bufs=8))
        opool = octx.enter_context(tc.tile_pool(name="opool", bufs=4))

        # ---------------- G = xn^T xn, spilled per e-chunk -----------------
        for ec in range(DC):
            for fh in range(2):
                fsl = slice(fh * FREE, (fh + 1) * FREE)
                pt = psum.tile([P, FREE], F32, tag="ps")
                for lc in range(LC):
                    nc.tensor.matmul(
                        pt[:], xnt[:, lc, ec * P:(ec + 1) * P],
                        xnt[:, lc, fsl],
                        start=(lc == 0), stop=(lc == LC - 1))
                nc.vector.tensor_copy(gsb[:, ec, fsl], pt[:])
            nc.sync.dma_start(
                Gsh[bass.ds(svo, 1), ec * P:(ec + 1) * P, :].rearrange(
                    "s (c p) n -> p (s c) n", p=P),
                gsb[:, ec:ec + 1])

        # pair barrier #1: DRAM->DRAM token sample (RAW dep on the spills
        # via the dynamic-slot alias), then a tiny AllReduce
        nc.sync.dma_start(
            tok[:], Gsh[bass.ds(svo, 1), 0:1, 0:2]
            .rearrange("s c n -> c (s n)"))
        pair_barrier = nc.gpsimd.collective_compute(
            "AllReduce", mybir.AluOpType.add, replica_groups=PAIRS,
            ins=[tok], outs=[tok2])

        # peer partial-G: read per chunk, split across the two idle HWDGE
        # rings so all 8 land quickly once the barrier fires
        gps = []
        for ec in range(DC):
            gp = gpp.tile([P, 1, D], BF16, tag="gp", name=f"gp_{ec}")
            eng = nc.scalar  # gated reads stay off the sync ring
            rd = eng.dma_start(
                gp[:], Gsh[bass.ds(svp, 1), ec * P:(ec + 1) * P, :].rearrange(
                    "s (c p) n -> p (s c) n", p=P))
            add_dep_helper(rd.ins, pair_barrier.ins,
                           reason="peer G after pair barrier")
            gps.append(gp)

        # ---------------- AT[c,a] = sum_b Wk[b,c] Wq[b,a] ------------------
        # (runs while the G exchange completes).  The peer-G adds ride the
        # DVE between the back-half AT copies: each costs ~0.5us and is
        # gated only on its own peer-chunk DMA, so they finish with AT and
        # M starts without an exposed add chain.
        for cc in range(DC):
            for ah in range(2):
                asl = slice(ah * FREE, (ah + 1) * FREE)
                pt = psum.tile([P, FREE], F32, tag="ps")
                for bb in range(DC):
                    nc.tensor.matmul(
                        pt[:], wkt[:, bb, cc * P:(cc + 1) * P],
                        wqt[:, bb, asl],
                        start=(bb == 0), stop=(bb == DC - 1))
                nc.vector.tensor_copy(atsb[:, cc, asl], pt[:])

        # peer-G adds on DVE, emitted after the whole AT copy stream so a
        # late barrier can never block the AT copies / PSUM recycling.
        # Column-halves first: M's first four groups only read the low
        # columns, so they unblock after half the add work.
        for half in range(2):
            hsl = slice(half * FREE, (half + 1) * FREE)
            for ec in range(DC):
                nc.vector.tensor_add(gsb[:, ec, hsl], gsb[:, ec, hsl],
                                     gps[ec][:, 0, hsl])

        # xT reuses xn's buffer (WAR: waits for the last G matmul read)
        xtt = xbig_pool.tile([P, DC, LH], BF16, tag="xb")  # xT chunked
        for h in range(2):  # scalar ring: after peer-G reads; needed at out
            nc.scalar.dma_start(xtt[:, 4 * h:4 * (h + 1)],
                                chunked(xT)[:, 4 * h:4 * (h + 1)])

        # ------------- M[c, own d-half] = sum_f G[f,c] WvT'[f, d] ----------
        for cc in range(DC):
            pt = psum.tile([P, FREE], F32, tag="ps")
            for fc in range(DC):
                nc.tensor.matmul(
                    pt[:], gsb[:, fc, cc * P:(cc + 1) * P],
                    wvt[:, fc],
                    start=(fc == 0), stop=(fc == DC - 1))
            nc.vector.tensor_copy(msb[:, cc], pt[:])

        # ------------- N[a, own d-half] = sum_c AT[c,a] M[c,d] -------------
        for ac in range(DC):
            pt = psum.tile([P, FREE], F32, tag="ps")
            for cc in range(DC):
                nc.tensor.matmul(
                    pt[:], atsb[:, cc, ac * P:(ac + 1) * P],
                    msb[:, cc],
                    start=(cc == 0), stop=(cc == DC - 1))
            nc.vector.tensor_copy(nsb[:, ac, 0:FREE], pt[:])

        # N-half exchange: spill own half, barrier #2, read peer half into
        # the high columns of nsb (hidden under the out own-half matmuls)
        nc.sync.dma_start(
            Nsh[bass.ds(svo, 1), :, :].rearrange("s (c p) n -> p (s c) n",
                                                 p=P),
            nsb[:, :, 0:FREE])
        nc.sync.dma_start(
            tok3[:], Nsh[bass.ds(svo, 1), 0:1, 0:2]
            .rearrange("s c n -> c (s n)"))
        pair_barrier2 = nc.gpsimd.collective_compute(
            "AllReduce", mybir.AluOpType.add, replica_groups=PAIRS,
            ins=[tok3], outs=[tok4])
        # ---------------- out[l,d] = sum_a xT[a,l] N[a,d] ------------------
        # own d-half first (no peer dep), peer half second; stores ride the
        # scalar ring so the sync ring is quiet while the N-mesh runs
        def out_half(dh):
            dsl = slice(dh * FREE, (dh + 1) * FREE)
            for lt in range(LC):
                ob = opool.tile([P, FREE], F32, tag="ob")
                pt = psum.tile([P, FREE], F32, tag="ps")
                for ac in range(DC):
                    nc.tensor.matmul(
                        pt[:], xtt[:, ac, lt * P:(lt + 1) * P],
                        nsb[:, ac, dsl],
                        start=(ac == 0), stop=(ac == DC - 1))
                nc.vector.tensor_copy(ob[:], pt[:])
                nc.scalar.dma_start(out[lt * P:(lt + 1) * P, dsl], ob[:])

        nrd = nc.scalar.dma_start(
            nsb[:, :, FREE:D],
            Nsh[bass.ds(svp, 1), :, :].rearrange("s (c p) n -> p (s c) n",
                                                 p=P))
        add_dep_helper(nrd.ins, pair_barrier2.ins,
                       reason="peer N after pair barrier 2")
        out_half(0)
        out_half(1)

    nc.compile()
    return nc


_NC_CACHE = {}


def _get_nc():
    if "nc" not in _NC_CACHE:
        _NC_CACHE["nc"] = build_nc()
    return _NC_CACHE["nc"]


def run(inputs, trace=False):
    """Run the kernel on all 8 cores. Returns (full_output, BassKernelResults)."""
    x = np.asarray(inputs["x"], dtype=np.float32)
    Wq = np.asarray(inputs["Wq"], dtype=np.float32)
    Wk = np.asarray(inputs["Wk"], dtype=np.float32)
    Wv = np.asarray(inputs["Wv"], dtype=np.float32)

    inv_sqrt_d = np.float32(1.0 / np.sqrt(D))
    wq_h = np.ascontiguousarray(Wq.astype(bfloat16))
    wk_h = np.ascontiguousarray(Wk.astype(bfloat16))
    wvT_f = (Wv.T * inv_sqrt_d).astype(bfloat16)
    xb = x.astype(bfloat16)

    in_maps = []
    for c in range(N_CORES):
        b, h = c // 2, c % 2
        rows = slice(h * LH, (h + 1) * LH)
        own = slice(h * FREE, (h + 1) * FREE)
        in_maps.append({
            "xn": np.ascontiguousarray(xb[b, rows, :]),
            "xT": np.ascontiguousarray(xb[b].T[:, rows]),
            "slots": np.array([[h, 1 - h]], dtype=np.uint32),
            "wq": wq_h, "wk": wk_h,
            "wvT": np.ascontiguousarray(wvT_f[:, own]),
        })

    nc = _get_nc()
    res = run_bass_kernel_spmd(nc, in_maps, list(range(N_CORES)), trace=trace)

    full = np.empty((B, L, D), dtype=np.float32)
    for c in range(N_CORES):
        b, h = c // 2, c % 2
        dev = res.results[c]["out"]
        rows = slice(h * LH, (h + 1) * LH)
        full[b, rows, h * FREE:(h + 1) * FREE] = dev[:, 0:FREE]
        full[b, rows, (1 - h) * FREE:(2 - h) * FREE] = dev[:, FREE:D]
    return full, res


def kernel(**inputs):
    full, _ = run(inputs, trace=False)
    return full
